# revision 31
# baseline (speedup 1.0000x reference)
"""Bass/Trainium2 kernel for nn_BernoulliMixture.

Reference computation (fp32):
    h = leaky_relu(x @ W_i2h^T + b_i2h)              [4096, 1024]
    z = softmax(h @ W_h2z^T + b_h2z)                 [4096, 32]
    d = sigmoid((h @ W_h2d^T + b_h2d) -> [.., 32, 784])
    out = einsum('tk,tko->to', z, d)                 [4096, 784]

Sharding (8 cores, SPMD): data-parallel over tokens. Each core handles
512 tokens and the full 32-component mixture.

The d-matmul (98% of all PE work) runs in fp8 e4m3 with
perf_mode=DoubleRow: both operands are [128, 2, N] APs pairing two
128-row contraction chunks per instruction, so the PE streams 2
elements/cycle — half the bf16 streaming cycles. Scales (h x4, w x32)
are folded into the host-prepped bias (x128) and the sigmoid
activation's scale (1/128), so no extra rescale pass is needed.
Numerics: max rel err ~1.5e-2 vs the 2e-2 gate (validated in numpy).

On-chip layout is token-major [128 tokens (partitions), free]:
  - h^T fp8 pair tiles [128 j, 2, t] are the stationary operand,
  - d-logits land in PSUM as [128 t, win]; bias-add (Vector
    tensor_tensor vs a bf16 bias slab), sigmoid (Scalar, scale=1/128),
    and the per-partition-scalar K-reduction (scalar_tensor_tensor:
    U += E_k * D) need no partition reductions.
  - for a subset of windows (PEB_WINS) the bias rides a rank-1 fp8
    matmul into PSUM instead of a Vector tensor_tensor, balancing
    Vector against the fp8 PE time.
"""

import os
from contextlib import ExitStack

import numpy as np

# ---------------------------------------------------------------------------
# problem constants (hardcoded; kernel.py must be self-contained)
B, L, IN, HID, K, O = 4, 1024, 512, 1024, 32, 784
N_CORES = 8
TOK_GROUPS = 8          # token-parallel
T = (B * L) // TOK_GROUPS          # 512 tokens per core
R = K * O                           # 25088 d-columns per core
TCHUNKS = T // 128                  # 4
JC = HID // 128                     # 8 contraction chunks of h
JP = JC // 2                        # 4 DoubleRow chunk pairs
IC = IN // 128                      # 4 contraction chunks of x

SH = 4.0                            # fp8 scale on h
SWD = 32.0                          # fp8 scale on w_h2d
INV_S = 1.0 / (SH * SWD)            # folded into the sigmoid activation

# d-matmul psum windows: 2048-wide (4 PSUM banks, 2 in flight) for the
# bulk, with a 512 taper so the PE->DVE pipeline drains with less backlog
WIN_PLAN = [2048] * 11 + [1024, 512, 512, 512]
assert sum(WIN_PLAN) == R
WIN_OFF = [sum(WIN_PLAN[:i]) for i in range(len(WIN_PLAN))]
N_WIN = len(WIN_PLAN)

SB = 1024.0                         # fp8 scale on b_h2d (PE-bias windows)
# windows whose bias rides a rank-1 fp8 matmul into PSUM (start=True on a
# (1/16*ones, b8) DoubleRow pair) instead of a Vector tensor_tensor --
# shifts bias-add work from the DVE to the PE to balance the two engines
PEB_WINS = frozenset((2, 4, 6, 8, 10, 11, 12))
N_WARMUP_MM = 8                    # garbage matmuls at t=0 to lift the HAM
                                    # clock gate before real work arrives

_PROGRAM = None


def _install_ldw_opt_patch():
    """bass_utils hardcodes --enable-ldw-opt=false, which makes walrus emit
    one serialized LDWEIGHTS per matmul (~170ns each on the PE critical
    path for DoubleRow's 256-column loads). Enable the optimizer so
    repeated stationary loads dedup/overlap."""
    import concourse.bass_utils as bu

    if getattr(bu, "_ldw_opt_patch_installed", False):
        return
    orig = bu.run_command

    def run_command_ldw(argv, **kwargs):
        argv = ["--enable-ldw-opt=true" if a == "--enable-ldw-opt=false"
                else a for a in argv]
        return orig(argv, **kwargs)

    bu.run_command = run_command_ldw
    bu._ldw_opt_patch_installed = True


def _install_drain_patch():
    """This image's walrus accepts at most ONE sync wait on CTRL-class
    instructions (Drain/NoOp). Stock Tile puts one wait per outstanding
    semaphore on the kernel-tail drain; split the extras into a chain of
    single-wait NOPs."""
    import concourse.tile as tile
    import concourse.mybir as mybir

    if getattr(tile.TileContext, "_drain_patch_installed", False):
        return

    def _drain_and_barrier(self, tick_clock, wait_clock):
        nc = self.nc
        drain_inst = nc.sync.drain()
        wait_clock.add_sem_waits(
            drain_inst.ins, tile.ScopedClock({None: tick_clock.global_clock})
        )
        si = drain_inst.ins.sync_info
        waits = list(si.on_wait or []) if si is not None else []
        if len(waits) > 1:
            si.on_wait = waits[:1]
            for w in waits[1:]:
                nop = nc.sync.nop()
                nop.ins.sync_info = mybir.SyncInfo(on_wait=[w], on_update=[])

        nc.all_engine_barrier()
        assert self.sems is not None
        popped = nc._tile_sem_poison_stack.pop()
        assert popped is self._sem_poison
        nc.clear_and_free_semaphores(list(self.sems.allocated().values()))
        nc.all_engine_barrier()

    tile.TileContext._drain_and_barrier = _drain_and_barrier
    tile.TileContext._drain_patch_installed = True


def _legalize_waits(nc):
    """This image's walrus accepts at most ONE sync wait per instruction.
    Hoist extra waits into preceding single-wait NOPs on the same engine
    (engines execute their stream in order, so a prior NOP-wait gates the
    instruction identically)."""
    import concourse.mybir as mybir

    n = 0
    for bass_bb in nc.bb_map.values():
        insts = bass_bb.bb.instructions
        i = 0
        while i < len(insts):
            inst = insts[i]
            si = inst.sync_info
            waits = list(si.on_wait) if si is not None and si.on_wait else []
            if len(waits) > 1:
                for w in waits[:-1]:
                    nop = mybir.InstNoOp(
                        name=f"waitnop_{n}", engine=inst.engine, ins=[], outs=[],
                        sync_info=mybir.SyncInfo(on_wait=[w], on_update=[]),
                    )
                    n += 1
                    insts.insert(i, nop)
                    i += 1
                si.on_wait = waits[-1:]
            i += 1
    return n


def _d_segments(w0, w1):
    """(kk, s0, s1) pieces of dram-column range [w0, w1) split at component
    boundaries (784 columns per component)."""
    segs = []
    for kk in range(w0 // O, (w1 - 1) // O + 1):
        s0, s1 = max(w0, kk * O), min(w1, (kk + 1) * O)
        segs.append((kk, s0, s1))
    return segs


def _build_program():
    import concourse.bass as bass
    import concourse.mybir as mybir
    import concourse.tile as tile

    _install_drain_patch()
    f32 = mybir.dt.float32
    f16 = mybir.dt.float16
    bf16 = mybir.dt.bfloat16
    f8 = mybir.dt.float8e4
    AF = mybir.ActivationFunctionType
    ALU = mybir.AluOpType
    DR = mybir.MatmulPerfMode.DoubleRow

    nc = bass.Bass("TRN2", target_bir_lowering=False, debug=False,
                   num_devices=N_CORES)

    d_xT = nc.dram_tensor("xT", [IC, 128, T], bf16, kind="ExternalInput").ap()
    d_wi2hT = nc.dram_tensor("wi2hT", [IC, 128, HID], bf16, kind="ExternalInput").ap()
    d_bi2h = nc.dram_tensor("bi2h", [128, JC], f32, kind="ExternalInput").ap()
    d_bneg = nc.dram_tensor("bneg", [128, JC], f32, kind="ExternalInput").ap()
    d_wzT = nc.dram_tensor("wzT", [128, JC, K], bf16, kind="ExternalInput").ap()
    d_bz = nc.dram_tensor("bz", [1, K], f32, kind="ExternalInput").ap()
    d_wdT = nc.dram_tensor("wdT", [128, JP, 2, R], f8, kind="ExternalInput").ap()
    d_bd = nc.dram_tensor("bd", [128, R], f32, kind="ExternalInput").ap()
    d_b8 = nc.dram_tensor("b8", [1, 2, R], f8, kind="ExternalInput").ap()
    d_ones8 = nc.dram_tensor("ones8", [1, 2, 128], f8, kind="ExternalInput").ap()
    d_out = nc.dram_tensor("out", [T, O], f32, kind="ExternalOutput").ap()

    with tile.TileContext(nc) as tc:
        with (
            tc.tile_pool(name="consts", bufs=1) as consts,
            tc.tile_pool(name="hpool", bufs=1) as hpool,
            tc.tile_pool(name="upool", bufs=1) as upool,
            tc.tile_pool(name="epool", bufs=1) as epool,
            tc.tile_pool(name="tmp", bufs=2) as tmp,
        ):

            # ---- warm-up: lift the HAM clock gate while input DMAs land ---
            warm_ctx = ExitStack()
            warm_psum = warm_ctx.enter_context(
                tc.tile_pool(name="warm_psum", bufs=1, space="PSUM"))
            wu_sb = consts.tile([128, 512], bf16, name="warm")
            nc.vector.memset(wu_sb[:], 0.0)
            wu_ps = warm_psum.tile([128, 512], f32, name="warm_ps")
            for i in range(N_WARMUP_MM):
                nc.tensor.matmul(
                    wu_ps[:], lhsT=wu_sb[:, 0:128], rhs=wu_sb[:],
                    start=True, stop=True, skip_group_check=True,
                )
            warm_ctx.close()

            # ---- phase H: h^T[j, t] = leaky_relu(x W^T + b) ----------------
            h_sb = [hpool.tile([128, T], bf16, tag=f"h{j}", name=f"h{j}")
                    for j in range(JC)]
            h8 = [hpool.tile([128, 2, T], f8, tag=f"h8_{jp}", name=f"h8_{jp}")
                  for jp in range(JP)]
            hzctx = ExitStack()
            hz_psum = hzctx.enter_context(
                tc.tile_pool(name="hz_psum", bufs=4, space="PSUM"))
            esc_sb = [None] * TCHUNKS
            dctx = ExitStack()
            wslab_pool = dctx.enter_context(tc.tile_pool(name="wslab", bufs=3))
            bslab_pool = dctx.enter_context(tc.tile_pool(name="bslab", bufs=2))
            dtmp = dctx.enter_context(tc.tile_pool(name="dtmp", bufs=3))

            def load_slabs(w):
                w0 = WIN_OFF[w]
                win = WIN_PLAN[w]
                w1 = w0 + win
                wsl = wslab_pool.tile([128, JP, 2, win], f8, tag="w",
                                      name=f"wsl{w}")
                for jp in range(JP):
                    nc.sync.dma_start(wsl[:, jp, :, :],
                                      d_wdT[:, jp, :, w0:w1])
                if w in PEB_WINS:
                    bsl = bslab_pool.tile([1, 2, win], f8, tag="b8",
                                          name=f"b8sl{w}")
                    nc.sync.dma_start(bsl[:], d_b8[:, :, w0:w1])
                else:
                    bsl = bslab_pool.tile([128, win], f32, tag="b",
                                          name=f"bsl{w}")
                    half = win // 2
                    nc.sync.dma_start(bsl[:, 0:half], d_bd[:, w0:w0 + half])
                    nc.sync.dma_start(bsl[:, half:win], d_bd[:, w0 + half:w1])
                return wsl, bsl

            with (
                tc.tile_pool(name="xw", bufs=1) as xw,
            ):
                x_sb, wi_sb = [], []
                for i in range(IC):
                    xt = xw.tile([128, T], bf16, tag=f"x{i}", name=f"x{i}")
                    x_sb.append(xt)
                    wt = xw.tile([128, HID], bf16, tag=f"wi{i}", name=f"wi{i}")
                    wi_sb.append(wt)
                # split the loads so the first matmul's operands land first
                # (one dma_start = one HW queue; fine pieces spread queues)
                for i in range(IC):
                    nc.sync.dma_start(wi_sb[i][:, 0:128], d_wi2hT[i][:, 0:128])
                    nc.scalar.dma_start(x_sb[i][:, 0:256], d_xT[i][:, 0:256])
                    nc.sync.dma_start(x_sb[i][:, 256:512], d_xT[i][:, 256:512])
                # constants ride the scalar-engine DMA queues, off the
                # critical SP dispatch path
                bi2h_sb = consts.tile([128, JC], f32)
                nc.scalar.dma_start(bi2h_sb[:], d_bi2h[:])
                bneg_sb = consts.tile([128, JC], f32)
                nc.scalar.dma_start(bneg_sb[:], d_bneg[:])
                wz_sb = consts.tile([128, JC, K], bf16)
                nc.scalar.dma_start(wz_sb[:], d_wzT[:])
                bz_sb = consts.tile([1, K], f32)
                nc.scalar.dma_start(bz_sb[:], d_bz[:])
                ones_sb = consts.tile([1, 128], f32)
                nc.vector.memset(ones_sb[:], 1.0)
                ones8_sb = consts.tile([1, 2, 128], f8)
                nc.scalar.dma_start(ones8_sb[:], d_ones8[:])
                u_sb = []
                for t in range(TCHUNKS):
                    u = upool.tile([128, O], f32, tag=f"u{t}", name=f"u{t}")
                    nc.vector.memset(u[:], 0.0)
                    u_sb.append(u)
                for i in range(IC):
                    for n4, (c0, c1) in enumerate(((128, 512), (512, 768),
                                                   (768, HID))):
                        eng = nc.scalar if n4 % 2 else nc.sync
                        eng.dma_start(wi_sb[i][:, c0:c1], d_wi2hT[i][:, c0:c1])
                    if T > 512:
                        nc.scalar.dma_start(x_sb[i][:, 512:T],
                                            d_xT[i][:, 512:T])
                preloaded = {w: load_slabs(w) for w in range(2)}

                # H and Z interleaved: after each 512-token half of h is
                # done, immediately compute that half's softmax numerators
                for tw in range(T // 512):
                    for j in range(JC):
                        ph = hz_psum.tile([128, 512], f32, tag="ph")
                        for i in range(IC):
                            nc.tensor.matmul(
                                ph[:],
                                lhsT=wi_sb[i][:, j * 128:(j + 1) * 128],
                                rhs=x_sb[i][:, tw * 512:(tw + 1) * 512],
                                start=(i == 0),
                                stop=(i == IC - 1),
                            )
                        r1 = xw.tile([128, 512], f32, tag="r1", bufs=2,
                                     name=f"r1_{tw}_{j}")
                        nc.scalar.activation(r1[:], ph[:], AF.Relu,
                                             bias=bi2h_sb[:, j:j + 1], scale=1.0)
                        r2 = xw.tile([128, 512], f32, tag="r2", bufs=2,
                                     name=f"r2_{tw}_{j}")
                        nc.scalar.activation(r2[:], ph[:], AF.Relu,
                                             bias=bneg_sb[:, j:j + 1], scale=-1.0)
                        # h = r1 - 0.01*r2  (leaky relu)
                        nc.vector.scalar_tensor_tensor(
                            out=h_sb[j][:, tw * 512:(tw + 1) * 512],
                            in0=r2[:], scalar=-0.01, in1=r1[:],
                            op0=ALU.mult, op1=ALU.add,
                        )
                        # fp8 copy (scaled by SH) in DoubleRow pair layout
                        nc.scalar.activation(
                            h8[j // 2][:, j % 2, tw * 512:(tw + 1) * 512],
                            h_sb[j][:, tw * 512:(tw + 1) * 512],
                            AF.Copy, scale=SH)
                    for t in range(tw * 4, tw * 4 + 4):
                        pz = hz_psum.tile([128, K], f32, tag="pz",
                                          name=f"pz{t}")
                        for j in range(JC):
                            nc.tensor.matmul(
                                pz[:],
                                lhsT=h_sb[j][:, t * 128:(t + 1) * 128],
                                rhs=wz_sb[:, j, :],
                                start=(j == 0),
                                stop=False,
                            )
                        # + b_h2z via rank-1 update: ones[t] x bz
                        nc.tensor.matmul(
                            pz[:],
                            lhsT=ones_sb[:],
                            rhs=bz_sb[:],
                            start=False,
                            stop=True,
                        )
                        e_t = epool.tile([128, K], f32, tag=f"e{t}",
                                         name=f"e{t}")
                        s_t = tmp.tile([128, 1], f32, tag="s", name=f"s{t}")
                        nc.scalar.activation(e_t[:], pz[:], AF.Exp,
                                             accum_out=s_t[:])
                        sinv = tmp.tile([128, 1], f32, tag="sinv",
                                        name=f"sinv{t}")
                        nc.vector.reciprocal(sinv[:], s_t[:])
                        esc = epool.tile([128, K], f32, tag=f"esc{t}",
                                         name=f"esc{t}")
                        nc.vector.tensor_scalar(esc[:], e_t[:], sinv[:], None,
                                                ALU.mult)
                        esc_sb[t] = esc

            # ---- phase D: stream W shard (fp8 DoubleRow), accumulate U -----
            hzctx.close()
            pctx = ExitStack()
            d_psum = pctx.enter_context(
                tc.tile_pool(name="d_psum", bufs=2, space="PSUM"))
            def emit_stt(pend):
                """Deferred K-reduction: emitted one tile late so the
                Vector queue never head-of-line blocks on this tile's
                sigmoid (the next tile's bias tensor_tensor is already
                enqueued ahead of it)."""
                t, ds, w0, segs = pend
                for kk, s0, s1 in segs:
                    nc.vector.scalar_tensor_tensor(
                        out=u_sb[t][:, s0 - kk * O:s1 - kk * O],
                        in0=ds[:, s0 - w0:s1 - w0],
                        scalar=esc_sb[t][:, kk:kk + 1],
                        in1=u_sb[t][:, s0 - kk * O:s1 - kk * O],
                        op0=ALU.mult, op1=ALU.add,
                    )

            pending = None
            for w in range(N_WIN):
                w0 = WIN_OFF[w]
                win = WIN_PLAN[w]
                w1 = w0 + win
                wsl, bsl = preloaded.pop(w) if w in preloaded else load_slabs(w)
                segs = _d_segments(w0, w1)
                nsub = win // 512
                t_order = range(TCHUNKS)
                if w >= N_WIN - 2:
                    t_order = reversed(range(TCHUNKS))
                peb = w in PEB_WINS
                for t in t_order:
                    pd = d_psum.tile([128, win], f32, tag="pd", name=f"pd{w}_{t}")
                    # jp outer / sub inner: all subs reuse the same
                    # stationary h pair, amortizing weight loads
                    if peb:
                        for sub in range(nsub):
                            nc.tensor.matmul(
                                pd[:, sub * 512:(sub + 1) * 512],
                                lhsT=ones8_sb[:],
                                rhs=bsl[:, :, sub * 512:(sub + 1) * 512],
                                start=True,
                                stop=False,
                                perf_mode=DR,
                            )
                    for jp in range(JP):
                        for sub in range(nsub):
                            nc.tensor.matmul(
                                pd[:, sub * 512:(sub + 1) * 512],
                                lhsT=h8[jp][:, :, t * 128:(t + 1) * 128],
                                rhs=wsl[:, jp, :, sub * 512:(sub + 1) * 512],
                                start=(jp == 0 and not peb),
                                stop=(jp == JP - 1),
                                perf_mode=DR,
                            )
                    if peb:
                        sig_in = pd
                    else:
                        sig_in = dtmp.tile([128, win], f32, tag="db")
                        nc.vector.tensor_tensor(sig_in[:], pd[:], bsl[:],
                                                ALU.add)
                    ds = dtmp.tile([128, win], f32, tag="ds")
                    nc.scalar.activation(ds[:], sig_in[:], AF.Sigmoid,
                                         scale=INV_S)
                    if pending is not None:
                        emit_stt(pending)
                    pending = (t, ds, w0, segs)
            if pending is not None:
                emit_stt(pending)

            for t in reversed(range(TCHUNKS)):
                nc.scalar.dma_start(d_out[t * 128:(t + 1) * 128, 0:392],
                                  u_sb[t][:, 0:392])
                nc.scalar.dma_start(d_out[t * 128:(t + 1) * 128, 392:O],
                                  u_sb[t][:, 392:O])
            pctx.close()
            dctx.close()

    _legalize_waits(nc)
    return nc


def _get_program():
    global _PROGRAM
    if _PROGRAM is None:
        _PROGRAM = _build_program()
    return _PROGRAM


def _prep_inputs(input, w_i2h, b_i2h, w_h2z, b_h2z, w_h2d, b_h2d):
    """Build the 8 per-core in_maps (host-side transposes/shards)."""
    import ml_dtypes
    f8 = ml_dtypes.float8_e4m3
    x_flat = np.ascontiguousarray(input.reshape(B * L, IN).astype(np.float32))
    wi2hT = np.ascontiguousarray(
        w_i2h.astype(np.float32).T.reshape(IC, 128, HID)
    ).astype(ml_dtypes.bfloat16)
    bi = np.ascontiguousarray(b_i2h.astype(np.float32).reshape(JC, 128).T)
    bn = np.ascontiguousarray(-bi)

    wz = np.ascontiguousarray(
        w_h2z.astype(np.float32).T.reshape(JC, 128, K).transpose(1, 0, 2)
    ).astype(ml_dtypes.bfloat16)
    bz = np.ascontiguousarray(b_h2z.astype(np.float32).reshape(1, K))

    # w_h2d^T in DoubleRow pair layout [128, JP, 2, R], fp8 e4m3, scaled
    wdT = w_h2d.astype(np.float32).T * np.float32(SWD)     # [HID, R]
    np.clip(wdT, -240.0, 240.0, out=wdT)
    wd = np.ascontiguousarray(
        wdT.reshape(JP, 2, 128, R).transpose(2, 0, 1, 3)
    ).astype(f8)
    bd = np.ascontiguousarray(np.broadcast_to(
        (b_h2d.astype(np.float32) * np.float32(SH * SWD)), (128, R)
    ).astype(np.float32))
    # PE-bias path: psum += 2 * (1/16) * (b*1024) = 128 * b
    b8 = np.ascontiguousarray(np.broadcast_to(
        (b_h2d.astype(np.float32) * np.float32(SB)), (1, 2, R)
    )).astype(f8)
    ones8 = np.full((1, 2, 128), SH * SWD / (2.0 * SB), dtype=f8)

    in_maps = []
    for core in range(N_CORES):
        tg = core
        xT = np.ascontiguousarray(
            x_flat[tg * T:(tg + 1) * T, :].T.reshape(IC, 128, T)
        ).astype(ml_dtypes.bfloat16)
        in_maps.append({
            "xT": xT, "wi2hT": wi2hT, "bi2h": bi, "bneg": bn,
            "wzT": wz, "bz": bz, "wdT": wd, "bd": bd,
            "b8": b8, "ones8": ones8,
        })
    return in_maps


LAST_RESULT = None


def kernel(**inputs):
    from concourse.bass_utils import run_bass_kernel_spmd

    global LAST_RESULT
    nc = _get_program()
    in_maps = _prep_inputs(**inputs)
    trace = bool(os.environ.get("BASS_KERNEL_TRACE"))
    if trace:
        try:
            _install_profile_shim()
        except Exception as e:  # degrade to untraced run
            print(f"profile shim unavailable ({e}); running untraced")
            trace = False
    res = run_bass_kernel_spmd(nc, in_maps, list(range(N_CORES)), trace=trace)
    LAST_RESULT = res

    out = np.empty((B * L, O), dtype=np.float32)
    for tg in range(TOK_GROUPS):
        out[tg * T:(tg + 1) * T] = np.asarray(
            res.results[tg]["out"]).astype(np.float32)
    return out.reshape(B, L, O)


def _install_profile_shim():
    """Register the NTFF profile hook concourse expects under axon (the
    image's antenv lacks axon_hooks) and stub the artifact upload."""
    import sys
    import types

    if "antenv.axon_hooks" not in sys.modules:
        from trn_agent_boot.trn_boot import _ntff_profile_via_ctypes

        hook = _ntff_profile_via_ctypes("/opt/axon/libaxon_pjrt.so")
        m = types.ModuleType("antenv.axon_hooks")
        m.get_axon_ntff_profile_hook = lambda: hook
        m.set_axon_ntff_profile_hook = lambda h: None
        sys.modules["antenv.axon_hooks"] = m

    import concourse.bass_utils as bu

    bu.upload_artifacts = lambda tmpdir: f"local://{tmpdir}"
